# revision 13
# baseline (speedup 1.0000x reference)
"""Trainium2 Bass kernel for nn_AttentionBlock_9792525435528.

Reference computation (per batch element b):
    xf = x[b].reshape(C, T)                      # C=512, T=32*32=1024
    GroupNorm(G=32) -> xn
    qkv = qkv_w @ xn + qkv_b                     # [3C, T]
    per head h (NH=8, ch=64): q,k,v; w = softmax((q*s)^T (k*s)); a = v @ w^T
    h = proj_w @ a + proj_b
    out = (xf + h) / sqrt(2)

Sharding: data-parallel over batch. 8 batch elements -> 8 NeuronCores, one
each. Weights replicated. No cross-core communication needed.

Device algorithm highlights:
  - float32r matmuls (full-rate PE, no data conversion; inputs stay fp32).
  - GroupNorm stats: bn_stats/bn_aggr per channel, then two tiny PE matmuls
    against constant indicator matrices to reduce across the 16 channels of
    each group and broadcast (mu, rstd) back to per-channel partitions.
  - Attention computed in the w^T[s,t] layout (softmax dim s on partitions):
    no max-subtraction needed (logits are O(1) by construction), exp runs on
    ScalarE straight out of PSUM into bf16 SBUF tiles, and the softmax
    denominator comes from an all-ones lhsT matmul col-tiled next to the
    a-matmul (both accumulate over s concurrently in different PE column
    groups). Division by the denominator uses reciprocal_approx_fast.
  - V is produced already transposed (v^T[s, c]) by swapping the matmul
    operands (lhsT = xn), so no on-device transpose is ever required.
  - q/k scale (1/sqrt(sqrt(ch))) and the final 1/sqrt(2) are folded into the
    weights on the host.
"""

import ml_dtypes
import numpy as np

import concourse.bass as bass
import concourse.mybir as mybir
import concourse.tile as tile
from concourse import bacc
from concourse.bass_utils import run_bass_kernel_spmd

B, C, T = 8, 512, 1024
NH, CH, G = 8, 64, 32
GS = C // G  # 16 channels per group
EPS = 1e-6
NCORES = 8
P = 128
KC = C // P  # 4 chunks of 128 input channels
SCN = T // P  # 8 s-chunks
NT = T // 512  # 2 t-chunks of 512
ISQ2 = float(1.0 / np.sqrt(2.0))
QK_SCALE = float(1.0 / np.sqrt(np.sqrt(CH)))

F32 = mybir.dt.float32
F32R = mybir.dt.float32r
BF16 = mybir.dt.bfloat16

_GRAPH_CACHE = {}


def _build_graph(qkv_bias_nz: bool, proj_bias_nz: bool, use_f32r: bool = True,
                 debug_taps: bool = False):
    nc = bacc.Bacc("TRN2", target_bir_lowering=False, debug=False)
    # All large matmuls run in bf16: 1 cycle/row on the PE (fp32/f32r run at
    # 2 cyc/row via the fp32_mode=HIGH path) plus fast weight load. PSUM
    # accumulation stays fp32; the residual path stays fp32 end-to-end, so
    # the bf16 rounding lands well inside the 2e-2 tolerance.
    MMD = BF16

    # ---- DRAM I/O ------------------------------------------------------
    x_d = nc.dram_tensor("x", [C, T], F32, kind="ExternalInput").ap()
    wq_d = nc.dram_tensor("wqT", [C, C], MMD, kind="ExternalInput").ap()
    wk_d = nc.dram_tensor("wkT", [C, C], MMD, kind="ExternalInput").ap()
    wv_d = nc.dram_tensor("wvT", [C, C], MMD, kind="ExternalInput").ap()
    pw_d = nc.dram_tensor("pwT", [C, C], MMD, kind="ExternalInput").ap()
    gnw_d = nc.dram_tensor("gnw", [C], F32, kind="ExternalInput").ap()
    gnb_d = nc.dram_tensor("gnb", [C], F32, kind="ExternalInput").ap()
    ind16_d = nc.dram_tensor("ind16", [C, G], F32, kind="ExternalInput").ap()
    indT_d = nc.dram_tensor("indT", [G, C], F32, kind="ExternalInput").ap()
    qb_d = kb_d = vb_d = pb_d = None
    if qkv_bias_nz:
        qb_d = nc.dram_tensor("qb", [C], F32, kind="ExternalInput").ap()
        kb_d = nc.dram_tensor("kb", [C], F32, kind="ExternalInput").ap()
        vb_d = nc.dram_tensor("vb", [C], F32, kind="ExternalInput").ap()
    if proj_bias_nz:
        pb_d = nc.dram_tensor("pb", [C], F32, kind="ExternalInput").ap()
    out_d = nc.dram_tensor("out", [C, T], F32, kind="ExternalOutput").ap()
    dbg = {}
    if debug_taps:
        MMDn = MMD
        dbg["xn"] = nc.dram_tensor("dbg_xn", [P, KC, T], MMDn, kind="ExternalOutput").ap()
        dbg["q"] = nc.dram_tensor("dbg_q", [P, KC, T], MMDn, kind="ExternalOutput").ap()
        dbg["k"] = nc.dram_tensor("dbg_k", [P, KC, T], MMDn, kind="ExternalOutput").ap()
        dbg["vt"] = nc.dram_tensor("dbg_vt", [P, SCN, NH * P], BF16, kind="ExternalOutput").ap()
        dbg["ew0"] = nc.dram_tensor("dbg_ew0", [SCN, P, T], BF16, kind="ExternalOutput").ap()
        dbg["a"] = nc.dram_tensor("dbg_a", [P, KC, T], MMDn, kind="ExternalOutput").ap()

    with tile.TileContext(nc) as tc:
        with (
            tc.tile_pool(name="big", bufs=1) as big,
            tc.tile_pool(name="wpool", bufs=1) as wpool,
            tc.tile_pool(name="small", bufs=1) as small,
            tc.tile_pool(name="ew", bufs=16) as ewpool,
            tc.tile_pool(name="rcp", bufs=3) as rcpool,
            tc.tile_pool(name="ps1", bufs=2, space="PSUM") as ps1,
            tc.tile_pool(name="ps2", bufs=2, space="PSUM") as ps2,
            tc.tile_pool(name="psg", bufs=2, space="PSUM") as psg,
        ):
            # ---- load inputs ------------------------------------------
            x_sb = big.tile([P, KC, T], F32, tag="x")
            nc.sync.dma_start(out=x_sb, in_=x_d.rearrange("(o p) t -> p o t", p=P))

            wq_sb = wpool.tile([P, KC, C], MMD, tag="wq")
            nc.sync.dma_start(out=wq_sb, in_=wq_d.rearrange("(o p) n -> p o n", p=P))
            wk_sb = wpool.tile([P, KC, C], MMD, tag="wk")
            nc.sync.dma_start(out=wk_sb, in_=wk_d.rearrange("(o p) n -> p o n", p=P))
            wv_sb = wpool.tile([P, KC, C], MMD, tag="wv")
            nc.sync.dma_start(out=wv_sb, in_=wv_d.rearrange("(o p) n -> p o n", p=P))
            pw_sb = wpool.tile([P, KC, C], MMD, tag="pw")
            nc.sync.dma_start(out=pw_sb, in_=pw_d.rearrange("(o p) n -> p o n", p=P))

            gnw_sb = small.tile([P, KC], F32, tag="gnw")
            nc.sync.dma_start(out=gnw_sb, in_=gnw_d.rearrange("(o p) -> p o", p=P))
            gnb_sb = small.tile([P, KC], F32, tag="gnb")
            nc.sync.dma_start(out=gnb_sb, in_=gnb_d.rearrange("(o p) -> p o", p=P))
            ind16_sb = small.tile([P, KC, G], F32, tag="ind16")
            nc.sync.dma_start(
                out=ind16_sb, in_=ind16_d.rearrange("(o p) g -> p o g", p=P)
            )
            indT_sb = small.tile([G, KC, P], F32, tag="indT")
            nc.sync.dma_start(out=indT_sb, in_=indT_d.rearrange("g (o p) -> g o p", p=P))

            bias_aps = {}
            for nm, d in (("qb", qb_d), ("kb", kb_d), ("pb", pb_d)):
                if d is not None:
                    t_ = small.tile([P, KC], F32, tag=nm)
                    nc.sync.dma_start(out=t_, in_=d.rearrange("(o p) -> p o", p=P))
                    bias_aps[nm] = t_
            if vb_d is not None:
                # v-bias varies along the free dim of v^T tiles: broadcast to
                # all 128 partitions once.
                vb_bc = small.tile([P, C], F32, tag="vb")
                nc.sync.dma_start(
                    out=vb_bc,
                    in_=bass.AP(tensor=vb_d.tensor, offset=vb_d.offset,
                                ap=[[0, P]] + vb_d.ap),
                )
                bias_aps["vb"] = vb_bc



            # ---- GroupNorm statistics ---------------------------------
            stats6 = small.tile([P, KC, 2, 6], F32, tag="stats6")
            mv = small.tile([P, KC, 2], F32, tag="mv")
            stats2 = small.tile([P, KC, 2], F32, tag="stats2")
            for o in range(KC):
                for hlf in range(2):
                    nc.vector.bn_stats(
                        out=stats6[:, o, hlf, :],
                        in_=x_sb[:, o, hlf * 512:(hlf + 1) * 512],
                    )
                nc.vector.bn_aggr(out=mv[:, o, :], in_=stats6[:, o, :, :])
                # stats2 = (mean, E[x^2]) per channel
                nc.vector.tensor_copy(out=stats2[:, o, 0:1], in_=mv[:, o, 0:1])
                nc.vector.tensor_mul(
                    out=stats2[:, o, 1:2], in0=mv[:, o, 0:1], in1=mv[:, o, 0:1]
                )
                nc.vector.tensor_add(
                    out=stats2[:, o, 1:2], in0=stats2[:, o, 1:2], in1=mv[:, o, 1:2]
                )

            # group reduce: psum_s[g, :] = (mu_g, E[x^2]_g)  (ind16 holds 1/16)
            psum_s = psg.tile([G, 2], F32, tag="psg")
            for k in range(KC):
                nc.tensor.matmul(
                    psum_s,
                    lhsT=ind16_sb[:, k, :],
                    rhs=stats2[:, k, :],
                    start=(k == 0),
                    stop=(k == KC - 1),
                )
            musd = small.tile([G, 2], F32, tag="musd")
            # musd[:,0] = mu ; musd[:,1] = rstd = exp(-0.5*ln(var+eps))
            nc.vector.tensor_copy(out=musd, in_=psum_s)  # (mu, E[x^2])
            varg = small.tile([G, 1], F32, tag="varg")
            nc.vector.tensor_mul(out=varg, in0=musd[:, 0:1], in1=musd[:, 0:1])
            nc.vector.tensor_sub(out=varg, in0=musd[:, 1:2], in1=varg)
            eps_sb = small.tile([G, 1], F32, tag="eps")
            nc.vector.memset(eps_sb, EPS)
            nc.scalar.activation(
                out=varg, in_=varg, func=mybir.ActivationFunctionType.Ln, bias=eps_sb
            )
            nc.scalar.activation(
                out=musd[:, 1:2], in_=varg,
                func=mybir.ActivationFunctionType.Exp, scale=-0.5,
            )

            # broadcast (mu, rstd) back to per-channel layout [P, KC, 2]
            musd_c = small.tile([P, KC, 2], F32, tag="musd_c")
            for o in range(KC):
                psum_b = psg.tile([P, 2], F32, tag="psg")
                nc.tensor.matmul(
                    psum_b, lhsT=indT_sb[:, o, :], rhs=musd, start=True, stop=True
                )
                nc.vector.tensor_copy(out=musd_c[:, o, :], in_=psum_b)

            # A = rstd * gn_w ; Bq = gn_b - mu * A   (per channel)
            A_sb = small.tile([P, KC], F32, tag="A")
            B_sb = small.tile([P, KC], F32, tag="B")
            for o in range(KC):
                nc.vector.tensor_mul(
                    out=A_sb[:, o:o + 1], in0=musd_c[:, o, 1:2], in1=gnw_sb[:, o:o + 1]
                )
                nc.vector.tensor_mul(
                    out=B_sb[:, o:o + 1], in0=musd_c[:, o, 0:1], in1=A_sb[:, o:o + 1]
                )
                nc.vector.tensor_sub(
                    out=B_sb[:, o:o + 1], in0=gnb_sb[:, o:o + 1], in1=B_sb[:, o:o + 1]
                )

            # xn = x * A + B
            xn_sb = big.tile([P, KC, T], MMD, tag="xn")
            for o in range(KC):
                nc.vector.tensor_scalar(
                    out=xn_sb[:, o, :], in0=x_sb[:, o, :],
                    scalar1=A_sb[:, o:o + 1], scalar2=B_sb[:, o:o + 1],
                    op0=mybir.AluOpType.mult, op1=mybir.AluOpType.add,
                )

            # ---- QKV projections --------------------------------------
            # q_sb/k_sb: [P, pair, T]; rows 0:64 = head 2j, 64:128 = head 2j+1
            q_sb = big.tile([P, KC, T], MMD, tag="q")
            k_sb = big.tile([P, KC, T], MMD, tag="k")
            for dst, w_sb, bias in (
                (q_sb, wq_sb, bias_aps.get("qb")),
                (k_sb, wk_sb, bias_aps.get("kb")),
            ):
                for j in range(KC):  # head pair
                    for t in range(NT):
                        pq = psg.tile([P, 512], F32, tag="psg")
                        for k in range(KC):
                            nc.tensor.matmul(
                                pq,
                                lhsT=w_sb[:, k, j * P:(j + 1) * P],
                                rhs=xn_sb[:, k, t * 512:(t + 1) * 512],
                                start=(k == 0),
                                stop=(k == KC - 1),
                            )
                        dslice = dst[:, j, t * 512:(t + 1) * 512]
                        if bias is not None:
                            nc.vector.tensor_scalar(
                                out=dslice, in0=pq, scalar1=bias[:, j:j + 1],
                                scalar2=None, op0=mybir.AluOpType.add,
                            )
                        else:
                            nc.vector.tensor_copy(out=dslice, in_=pq)

            # v^T augmented: per head 128 cols = [64 v^T cols | 64 ones].
            # MM2's lhsT is then [s, 128]: rows 0:64 of its PSUM output get
            # sum_s v^T*ew (attention numerator) and rows 64:128 get
            # sum_s ew (softmax denominator) in a single accumulation group.
            vT_sb = big.tile([P, SCN, NH * P], BF16, tag="vT")
            vT4 = vT_sb.rearrange("p s (h z) -> p s h z", z=P)
            nc.vector.memset(vT4[:, :, :, CH:P], 1.0)
            for sc in range(SCN):
                pv = psg.tile([P, 512], F32, tag="psg")
                for k in range(KC):
                    nc.tensor.matmul(
                        pv,
                        lhsT=xn_sb[:, k, sc * P:(sc + 1) * P],
                        rhs=wv_sb[:, k, :],
                        start=(k == 0),
                        stop=(k == KC - 1),
                    )
                vdst = vT4[:, sc, :, 0:CH]  # [P, NH, CH] strided dst
                if "vb" in bias_aps:
                    nc.vector.scalar_tensor_tensor(
                        out=vdst, in0=pv.rearrange("p (h z) -> p h z", z=CH),
                        scalar=0.0,
                        in1=bias_aps["vb"].rearrange("p (h z) -> p h z", z=CH),
                        op0=mybir.AluOpType.add, op1=mybir.AluOpType.add,
                    )
                else:
                    nc.vector.tensor_copy(
                        out=vdst, in_=pv.rearrange("p (h z) -> p h z", z=CH)
                    )

            if debug_taps:
                nc.sync.dma_start(out=dbg["xn"], in_=xn_sb)
                nc.sync.dma_start(out=dbg["q"], in_=q_sb)
                nc.sync.dma_start(out=dbg["k"], in_=k_sb)
                nc.sync.dma_start(out=dbg["vt"], in_=vT_sb)

            # ---- attention + proj rhs ---------------------------------
            a_sb = big.tile([P, KC, T], MMD, tag="a")
            for j in range(KC):  # head pairs (2j, 2j+1)
                ew = {}  # (hb, sc) -> bf16 [P, T] tile of exp(w^T)
                for sc in range(SCN):
                    ptiles = {}
                    for hb in range(2):  # row-group-tiled head pair
                        h0 = hb * CH
                        pw1 = ps1.tile([P, T], F32, tag="ps1")
                        for t in range(NT):
                            nc.tensor.matmul(
                                pw1[:, t * 512:(t + 1) * 512],
                                lhsT=k_sb[h0:h0 + CH, j, sc * P:(sc + 1) * P],
                                rhs=q_sb[h0:h0 + CH, j, t * 512:(t + 1) * 512],
                                start=True,
                                stop=True,
                            )
                        ptiles[hb] = pw1
                    for hb in range(2):
                        et = ewpool.tile([P, T], BF16, tag="ew")
                        nc.scalar.activation(
                            out=et, in_=ptiles[hb],
                            func=mybir.ActivationFunctionType.Exp,
                        )
                        ew[(hb, sc)] = et
                        if debug_taps and j == 0 and hb == 0:
                            nc.sync.dma_start(out=dbg["ew0"][sc], in_=et)

                for hb in range(2):
                    h = 2 * j + hb
                    for t in range(NT):
                        pa = ps2.tile([P, 512], F32, tag="ps2")
                        for sc in range(SCN):
                            # rows 0:64 <- sum_s v^T[s,c]*ew[s,t]
                            # rows 64:128 <- sum_s ew[s,t] (softmax denom)
                            nc.tensor.matmul(
                                pa,
                                lhsT=vT_sb[:, sc, h * P:(h + 1) * P],
                                rhs=ew[(hb, sc)][:, t * 512:(t + 1) * 512],
                                start=(sc == 0),
                                stop=(sc == SCN - 1),
                            )
                        # reciprocal_approx_fast cannot read PSUM (HW-
                        # verified): stage the denominator into SBUF first.
                        d_sb = rcpool.tile([CH, 512], F32, tag="dcp")
                        nc.vector.tensor_copy(out=d_sb, in_=pa[CH:2 * CH, :])
                        r_sb = rcpool.tile([CH, 512], F32, tag="rcp")
                        nc.vector.reciprocal_approx_fast(out=r_sb, in_=d_sb)
                        nc.vector.tensor_mul(
                            out=a_sb[hb * CH:(hb + 1) * CH, j,
                                     t * 512:(t + 1) * 512],
                            in0=pa[0:CH, :],
                            in1=r_sb,
                        )

            if debug_taps:
                nc.sync.dma_start(out=dbg["a"], in_=a_sb)

            # ---- output projection + residual -------------------------
            out_sb = big.tile([P, KC, T], F32, tag="osb")
            for o in range(KC):
                for t in range(NT):
                    ph = psg.tile([P, 512], F32, tag="psg")
                    for k in range(KC):
                        nc.tensor.matmul(
                            ph,
                            lhsT=pw_sb[:, k, o * P:(o + 1) * P],
                            rhs=a_sb[:, k, t * 512:(t + 1) * 512],
                            start=(k == 0),
                            stop=(k == KC - 1),
                        )
                    if "pb" in bias_aps:
                        nc.vector.tensor_scalar(
                            out=ph, in0=ph, scalar1=bias_aps["pb"][:, o:o + 1],
                            scalar2=None, op0=mybir.AluOpType.add,
                        )
                    # out = x * (1/sqrt2) + h'   (1/sqrt2 folded into pwT/pb)
                    nc.vector.scalar_tensor_tensor(
                        out=out_sb[:, o, t * 512:(t + 1) * 512],
                        in0=x_sb[:, o, t * 512:(t + 1) * 512],
                        scalar=ISQ2,
                        in1=ph,
                        op0=mybir.AluOpType.mult,
                        op1=mybir.AluOpType.add,
                    )

            nc.sync.dma_start(
                out=out_d.rearrange("(o p) t -> p o t", p=P), in_=out_sb
            )

    nc.compile()
    return nc


def _host_prep(qkv_w, qkv_b, proj_w, proj_b):
    """Build the replicated (per-core-identical) weight/const arrays."""
    qkv_w = np.asarray(qkv_w, np.float32)
    qkv_b = np.asarray(qkv_b, np.float32)
    proj_w = np.asarray(proj_w, np.float32)
    proj_b = np.asarray(proj_b, np.float32)

    w3 = qkv_w.reshape(NH, 3 * CH, C)  # per head: rows 0:64 q, 64:128 k, 128:192 v
    b3 = qkv_b.reshape(NH, 3 * CH)
    wq = w3[:, 0:CH, :] * QK_SCALE          # [NH, CH, C]
    wk = w3[:, CH:2 * CH, :] * QK_SCALE
    wv = w3[:, 2 * CH:3 * CH, :]
    qb = (b3[:, 0:CH] * QK_SCALE).reshape(C)
    kb = (b3[:, CH:2 * CH] * QK_SCALE).reshape(C)
    vb = b3[:, 2 * CH:3 * CH].reshape(C)

    # lhsT layouts [C_in, C_out-ish]: column r of pair-block j is head 2j's q
    # row r (r<64) or head 2j+1's q row r-64.
    BF = ml_dtypes.bfloat16
    wqT = np.ascontiguousarray(wq.reshape(C, C).T.astype(BF))  # [C_in, NH*CH]
    wkT = np.ascontiguousarray(wk.reshape(C, C).T.astype(BF))
    wvT = np.ascontiguousarray(wv.reshape(C, C).T.astype(BF))
    pwT = np.ascontiguousarray((proj_w * ISQ2).T.astype(BF))
    pb = proj_b * ISQ2

    # per-partition bias layouts for q/k ([C] ordered head-major == qkv order)
    ind16 = np.zeros((C, G), np.float32)
    ind16[np.arange(C), np.arange(C) // GS] = 1.0 / GS
    indT = np.zeros((G, C), np.float32)
    indT[np.arange(C) // GS, np.arange(C)] = 1.0

    return dict(
        wqT=wqT, wkT=wkT, wvT=wvT, pwT=pwT,
        qb=qb, kb=kb, vb=vb, pb=pb,
        ind16=ind16, indT=indT,
    )


def kernel(**inputs):
    x = np.asarray(inputs["x"], np.float32)
    gn_w = np.asarray(inputs["gn_w"], np.float32)
    gn_b = np.asarray(inputs["gn_b"], np.float32)
    qkv_b = np.asarray(inputs["qkv_b"], np.float32)
    proj_b = np.asarray(inputs["proj_b"], np.float32)

    prep = _host_prep(inputs["qkv_w"], qkv_b, inputs["proj_w"], proj_b)
    qkv_bias_nz = bool(np.any(qkv_b != 0))
    proj_bias_nz = bool(np.any(proj_b != 0))

    key = (qkv_bias_nz, proj_bias_nz)
    if key not in _GRAPH_CACHE:
        _GRAPH_CACHE[key] = _build_graph(qkv_bias_nz, proj_bias_nz)
    nc = _GRAPH_CACHE[key]

    shared = dict(
        wqT=prep["wqT"], wkT=prep["wkT"], wvT=prep["wvT"], pwT=prep["pwT"],
        gnw=gn_w, gnb=gn_b, ind16=prep["ind16"], indT=prep["indT"],
    )
    if qkv_bias_nz:
        shared.update(qb=prep["qb"], kb=prep["kb"], vb=prep["vb"])
    if proj_bias_nz:
        shared.update(pb=prep["pb"])

    in_maps = [
        {**shared, "x": np.ascontiguousarray(x[i].reshape(C, T))}
        for i in range(NCORES)
    ]
    res = run_bass_kernel_spmd(nc, in_maps, core_ids=list(range(NCORES)))
    out = np.stack(
        [res.results[i]["out"].reshape(C, 32, 32) for i in range(NCORES)]
    )
    kernel._last_results = res
    return out
